# revision 16
# baseline (speedup 1.0000x reference)
"""Multi-head self-attention + projector, Trainium2 Bass kernel, 8 NeuronCores.

Reference computation (per batch b):
    Q = X @ Wq + bq; K = X @ Wk + bk; V = X @ Wv + bv      (X: [S, D])
    per head h: P_h = softmax(Q_h K_h^T / sqrt(dh)); A_h = P_h V_h
    Y = concat_h(A_h) @ Wo + bo
Sharding: core i handles batch i//2, query rows (i%2)*1024 .. +1024.
K/V are computed for the full sequence on each core (no collectives).
The host rolls each core's query columns to the front of X^T so a single
SPMD program serves all 8 cores.

Algebraic simplifications (exact w.r.t. softmax):
  - bk dropped (per-query constant in scores, softmax cancels it)
  - bv folded into the output bias on host (softmax rows sum to 1)
  - no max-subtraction in softmax (scores are O(1) for these inputs)

The kernel is exp-throughput bound on the Scalar engine (ACT cost
(N+352)/1.2 ns), so the design maximizes ACT batch size and hides all
other work under the exp stream:
  - scores PSUM alternates a 3-bank and a 2-bank tile; each tile is
    exp'd by ONE ACTIVATE (N=1536/1024) -> 1088 ns/kt vs 1147 at N=1024.
  - attended pair is col-packed: h0 -> psum partitions 0:64 at
    tile_position (0,0), h1 -> 64:128 at (0,64), concurrently, in ONE
    bank.  Only the first matmul into the bank uses start=True (start
    clears has_written for the whole bank; later matmuls overwrite
    where unwritten, accumulate where written).
  - softmax row-sums via M=1 ones-matmuls, 4-way col-packed
    (positions 0/32/64/96) every 2 kt into one more bank.
  - PSUM: 3 (scores A) + 2 (scores B) + 1 (attended) + 1 (rowsum)
    + 1 (injected projection/output jobs) = 8 banks.
  - warmup matmuls on a zeroed tile during the input-DMA window keep
    the PE HAM clock-gate at 8/8 before real work lands.
  - phase A: only K-chunk0 / Q-chunk0 / V0..2 run up front; V3..15
    inject into iteration 1, K/Q chunks 1-3 trickle into later
    iterations (one chunk ahead of first use).
  - last two (qb,hp) iterations run as 256-query halves so the final
    Y = attended @ Wo jobs and their DMAs pipeline into the exp window.
"""

import numpy as np

import concourse.bass as bass
import concourse.mybir as mybir
import concourse.tile as tile
from concourse import bacc, bass_utils

F32 = mybir.dt.float32
BF16 = mybir.dt.bfloat16

B, S, D, HID, HEADS, DH, VD = 4, 2048, 768, 512, 8, 64, 768
N_CORES = 8
SQ = S // 2  # query rows per core
DC = D // 128  # 6 contraction chunks for the projections
HC = HID // 128  # 4 hidden chunks
KT = S // 128  # 16 key chunks
# per-iteration exp batching: 32 (kt, head) slots -> psum tiles of 3,2,3,...,2
BATCH_SIZES = [3, 2] * 6 + [2]
SLOT_MAP = []  # slot -> (batch_idx, sub_idx)
for _b, _sz in enumerate(BATCH_SIZES):
    for _i in range(_sz):
        SLOT_MAP.append((_b, _i))
assert len(SLOT_MAP) == 2 * KT


def _kernel_body(tc):
    nc = tc.nc
    xt_d = nc.dram_tensor("xt", [D, S], BF16, kind="ExternalInput").ap()
    wq_d = nc.dram_tensor("wq", [D, HID], BF16, kind="ExternalInput").ap()
    wk_d = nc.dram_tensor("wk", [D, HID], BF16, kind="ExternalInput").ap()
    wv_d = nc.dram_tensor("wv", [D, HID], BF16, kind="ExternalInput").ap()
    bq_d = nc.dram_tensor("bq", [HID], F32, kind="ExternalInput").ap()
    wo_d = nc.dram_tensor("wo", [HID, VD], BF16, kind="ExternalInput").ap()
    bo_d = nc.dram_tensor("bo2", [VD], F32, kind="ExternalInput").ap()
    y_d = nc.dram_tensor("y", [SQ, VD], F32, kind="ExternalOutput").ap()

    with (
        tc.tile_pool(name="persist", bufs=1) as persist,
        tc.tile_pool(name="scA", bufs=1, space="PSUM") as scA_pool,
        tc.tile_pool(name="scB", bufs=1, space="PSUM") as scB_pool,
        tc.tile_pool(name="att_ps", bufs=1, space="PSUM") as att_ps_pool,
        tc.tile_pool(name="rs_ps", bufs=1, space="PSUM") as rs_ps_pool,
        tc.tile_pool(name="inj_ps", bufs=1, space="PSUM") as inj_ps_pool,
        tc.tile_pool(name="pa_sbuf", bufs=1) as pa_sbuf,
        tc.tile_pool(name="eA_pool", bufs=4) as eA_pool,
        tc.tile_pool(name="eB_pool", bufs=4) as eB_pool,
        tc.tile_pool(name="rb_pool", bufs=2) as rb_pool,
        tc.tile_pool(name="tmp_pool", bufs=2) as tmp_pool,
        tc.tile_pool(name="y_sb", bufs=2) as y_sb_pool,
    ):
        # ---- persistent SBUF tensors ----
        wo_sb = persist.tile([128, HC, VD], BF16)
        bo_sb = persist.tile([128, VD], F32)
        bq_sb = persist.tile([128, HC], F32)
        qt_sb = persist.tile([128, HC, SQ], BF16)
        kt_sb = persist.tile([128, HC, S], BF16)
        v_sb = persist.tile([128, KT, HEADS, DH], BF16)
        att_sb = persist.tile([128, HC, SQ], BF16)
        zero_sb = persist.tile([128, 1], F32)
        ones_sb = persist.tile([128, 1], BF16)
        warm_sb = persist.tile([128, 128], BF16)

        nc.vector.memset(zero_sb[:], 0.0)
        nc.vector.memset(ones_sb[:], 1.0)
        nc.vector.memset(warm_sb[:], 0.0)
        nc.sync.dma_start(out=bq_sb[:], in_=bq_d.rearrange("(c p) -> p c", c=HC))

        # ---- PE warmup: keep the HAM clock-gate busy during input DMA ----
        # ~31 cold matmuls warm the clock (3.4us), the rest hold it at 8/8
        # until the first input chunks land (~12us).
        warm_ps = inj_ps_pool.tile([128, 512], F32, tag="inj")
        for i in range(170):
            nc.tensor.matmul(
                warm_ps[:, 0:128],
                warm_sb[:],
                warm_sb[:],
                start=True,
                stop=True,
            )

        xt_sb = pa_sbuf.tile([128, DC, S], BF16)
        wq_sb = pa_sbuf.tile([128, DC, HID], BF16)
        wk_sb = pa_sbuf.tile([128, DC, HID], BF16)
        wv_sb = pa_sbuf.tile([128, DC, HID], BF16)

        # input DMAs: xt on sync, weights split across gpsimd/vector so the
        # (exp-saturated) scalar queue never issues transfers
        xt_r = xt_d.rearrange("(c p) s -> c p s", c=DC)
        dma_engines = [nc.gpsimd, nc.sync, nc.gpsimd]
        for c in range(DC):
            nc.sync.dma_start(out=xt_sb[:, c, :], in_=xt_r[c])
            for e_i, (w_sb, w_d) in enumerate(
                ((wk_sb, wk_d), (wq_sb, wq_d), (wv_sb, wv_d))
            ):
                w_r = w_d.rearrange("(c p) h -> c p h", c=DC)
                dma_engines[e_i].dma_start(out=w_sb[:, c, :], in_=w_r[c])

        def load_wo_bo():
            for c in range(HC):
                nc.sync.dma_start(
                    out=wo_sb[:, c, :],
                    in_=wo_d.rearrange("(c p) v -> c p v", c=HC)[c],
                )
            bo_row = rb_pool.tile([1, VD], F32, tag="bo_row")
            nc.sync.dma_start(out=bo_row[0:1, :], in_=bo_d[None, :])
            nc.gpsimd.partition_broadcast(bo_sb[:], bo_row[0:1, :])

        # ---- phase A jobs (QKV projections), one [128,512] psum bank each ----
        def emit_pa_job(kind, a, b, ps, off, d_lo=0, d_hi=DC):
            for i in range(d_lo, d_hi):
                d = (off + i) % DC
                if kind == "q":
                    lhsT = wq_sb[:, d, a * 128 : (a + 1) * 128]
                    rhs = xt_sb[:, d, b * 512 : (b + 1) * 512]
                elif kind == "k":
                    lhsT = wk_sb[:, d, a * 128 : (a + 1) * 128]
                    rhs = xt_sb[:, d, b * 512 : (b + 1) * 512]
                else:
                    lhsT = xt_sb[:, d, a * 128 : (a + 1) * 128]
                    rhs = wv_sb[:, d, :]
                nc.tensor.matmul(ps, lhsT, rhs, start=(i == 0), stop=(i == DC - 1))
            if d_hi < DC:
                return
            if kind == "q":
                nc.vector.tensor_scalar_add(
                    out=qt_sb[:, a, b * 512 : (b + 1) * 512],
                    in0=ps,
                    scalar1=bq_sb[:, a : a + 1],
                )
            elif kind == "k":
                nc.vector.tensor_copy(
                    out=kt_sb[:, a, b * 512 : (b + 1) * 512], in_=ps
                )
            else:
                nc.vector.tensor_copy(
                    out=v_sb[:, a, :, :],
                    in_=ps.rearrange("p (h d) -> p h d", h=HEADS),
                )

        pa_count = [0]

        def emit_inj_job(job):
            ps = inj_ps_pool.tile([128, 512], F32, tag="inj")
            emit_pa_job(*job, ps[:], pa_count[0] % DC)
            pa_count[0] += 1

        def inj_thunk(job):
            def thunk():
                with tc.high_priority(offset=-60):
                    emit_inj_job(job)

            return thunk

        def inj_half_thunks(job):
            # one job as two 3-matmul halves sharing a psum tile, so each
            # injection point displaces scores by less than the PE slack
            state = {}

            def first():
                with tc.high_priority(offset=-60):
                    state["ps"] = inj_ps_pool.tile([128, 512], F32, tag="inj", name="inj_ps")
                    state["off"] = pa_count[0] % DC
                    pa_count[0] += 1
                    emit_pa_job(*job, state["ps"][:], state["off"], 0, DC // 2)

            def second():
                with tc.high_priority(offset=-60):
                    emit_pa_job(*job, state["ps"][:], state["off"], DC // 2, DC)

            return first, second

        # ---- phase C job: Y[qt] = attended^T.T @ Wo + bo, via the inject bank
        def emit_y(qt_i):
            y_sb = y_sb_pool.tile([128, VD], F32, tag="ysb")
            y_ps = inj_ps_pool.tile([128, 512], F32, tag="inj")
            for c in range(HC):
                lhsT = att_sb[:, c, qt_i * 128 : (qt_i + 1) * 128]
                nc.tensor.matmul(
                    y_ps[:], lhsT, wo_sb[:, c, 0:512], start=(c == 0), stop=(c == HC - 1)
                )
            nc.vector.tensor_add(y_sb[:, 0:512], y_ps[:], bo_sb[:, 0:512])
            y_ps2 = inj_ps_pool.tile([128, 512], F32, tag="inj")
            for c in range(HC):
                lhsT = att_sb[:, c, qt_i * 128 : (qt_i + 1) * 128]
                nc.tensor.matmul(
                    y_ps2[:, 0 : VD - 512],
                    lhsT,
                    wo_sb[:, c, 512:VD],
                    start=(c == 0),
                    stop=(c == HC - 1),
                )
            nc.vector.tensor_add(
                y_sb[:, 512:VD], y_ps2[:, 0 : VD - 512], bo_sb[:, 512:VD]
            )
            nc.sync.dma_start(
                out=y_d.rearrange("(t p) v -> t p v", p=128)[qt_i], in_=y_sb[:]
            )

        def y_thunk(qt_i):
            def thunk():
                with tc.high_priority(offset=-60):
                    emit_y(qt_i)

            return thunk

        # ---- phase B attention iteration ----
        # attended matmuls / rowsums / epilogue are deferred (pend) so the
        # next scores+exp always lead on the PE/ACT streams.
        pend = []
        period = [0]

        def flush_pend(lag=0):
            while pend and pend[0][0] <= period[0] - lag:
                pend.pop(0)[1]()

        def emit_attention(hp, qoff, qw, inject=None, lag=3):
            h0, h1 = 2 * (hp % HEADS), 2 * (hp % HEADS) + 1
            qs = qt_sb[:, hp, qoff : qoff + qw]
            att_ps = [None]
            rs_ps = [None]
            e_ref = {}  # slot -> (e_tile, sub_idx)
            s_tiles = {}

            def attended(kt):
                def thunk():
                    eh0, i0 = e_ref[2 * kt]
                    eh1, i1 = e_ref[2 * kt + 1]
                    if kt == 0:
                        att_ps[0] = att_ps_pool.tile([128, 512], F32, tag="att", name="att_ps")
                        # zero the bank, then accumulate with start=False
                        # throughout: correct whatever the has_written bits
                        # are (set -> 0+x, clear -> overwrite), and CoreSim's
                        # accumulate-onto-current model agrees.
                        nc.vector.memset(att_ps[0][:], 0.0)
                    att = att_ps[0]
                    # col-packed pair: h0 -> partitions 0:64 (tile 0,0),
                    # h1 -> 64:128 (tile 0,64), concurrently.
                    nc.tensor.matmul(
                        att[0:DH, 0:qw],
                        v_sb[:, kt, h0, :],
                        eh0[:, i0, 0:qw],
                        start=False,
                        stop=(kt == KT - 1),
                        skip_group_check=True,
                    )
                    nc.tensor.matmul(
                        att[DH:128, 0:qw],
                        v_sb[:, kt, h1, :],
                        eh1[:, i1, 0:qw],
                        start=False,
                        stop=(kt == KT - 1),
                        skip_group_check=True,
                    )
                    if kt % 2 == 1:
                        # rowsums for kts (kt-1, kt): 4 col-packed M=1
                        # matmuls at psum partitions 0/32/64/96
                        if kt == 1:
                            rs_ps[0] = rs_ps_pool.tile([128, 512], F32, tag="rs", name="rs_ps")
                            nc.vector.memset(rs_ps[0][:], 0.0)
                        rs = rs_ps[0]
                        quads = [
                            (0, e_ref[2 * kt - 2]),   # h0, even kt
                            (32, e_ref[2 * kt - 1]),  # h1, even kt
                            (64, e_ref[2 * kt]),      # h0, odd kt
                            (96, e_ref[2 * kt + 1]),  # h1, odd kt
                        ]
                        for pos, (et, ei) in quads:
                            nc.tensor.matmul(
                                rs[pos : pos + 1, 0:qw],
                                ones_sb[:],
                                et[:, ei, 0:qw],
                                start=False,
                                stop=(kt == KT - 1),
                                skip_group_check=True,
                                tile_position=(0, pos),
                            )

                return thunk

            def epilogue():
                att = att_ps[0]
                rs = rs_ps[0]
                # gather the 4 rowsum partials onto partition 0, stacked on
                # the free axis: g[0] = pos0(h0 even), g[1] = pos32(h1 even),
                # g[2] = pos64(h0 odd), g[3] = pos96(h1 odd) so the strided
                # add pairs (h0e+h0o, h1e+h1o)
                rs_sb = tmp_pool.tile([97, 512], F32, tag="rs_sb")
                nc.vector.tensor_copy(rs_sb[0:97, 0:qw], rs[0:97, 0:qw])
                g = tmp_pool.tile([1, 4, 512], F32, tag="g")
                nc.sync.dma_start(g[0:1, 0, 0:qw], rs_sb[0:1, 0:qw])
                nc.gpsimd.dma_start(g[0:1, 1, 0:qw], rs_sb[32:33, 0:qw])
                nc.sync.dma_start(g[0:1, 2, 0:qw], rs_sb[64:65, 0:qw])
                nc.gpsimd.dma_start(g[0:1, 3, 0:qw], rs_sb[96:97, 0:qw])
                # rec[0,0,:] = 1/rowsum_h0, rec[0,1,:] = 1/rowsum_h1
                rec = rb_pool.tile([1, 2, 512], F32, tag="rec")
                nc.vector.tensor_add(
                    rec[0:1, :, 0:qw], g[0:1, 0:2, 0:qw], g[0:1, 2:4, 0:qw]
                )
                nc.vector.reciprocal_approx_fast(rec[0:1, :, 0:qw], rec[0:1, :, 0:qw])
                rb = rb_pool.tile([128, 512], F32, tag="rb")
                rbB = rb_pool.tile([64, 512], F32, tag="rbB")
                nc.gpsimd.partition_broadcast(rb[0:DH, 0:qw], rec[0:1, 0, 0:qw])
                nc.gpsimd.partition_broadcast(rbB[0:DH, 0:qw], rec[0:1, 1, 0:qw])
                # partition-shift the h1 reciprocals into rb's top half
                nc.sync.dma_start(rb[DH:128, 0:qw], rbB[0:DH, 0:qw])
                nc.vector.tensor_mul(
                    att_sb[:, hp, qoff : qoff + qw], att[:, 0:qw], rb[:, 0:qw]
                )

            for kt in range(KT):
                if inject and kt in inject:
                    inject[kt]()
                ks = kt_sb[:, hp, kt * 128 : (kt + 1) * 128]
                for h in (0, 1):
                    slot = 2 * kt + h
                    b_idx, sub = SLOT_MAP[slot]
                    sz = BATCH_SIZES[b_idx]
                    if sub == 0:
                        pool = scA_pool if sz == 3 else scB_pool
                        s_tiles[b_idx] = pool.tile(
                            [128, sz, 512], F32, tag="s", name="s_ps"
                        )
                    s_ps = s_tiles[b_idx]
                    nc.tensor.matmul(
                        s_ps[:, sub, 0:qw],
                        ks[64 * h : 64 * h + 64, :],
                        qs[64 * h : 64 * h + 64, :],
                        start=True,
                        stop=True,
                    )
                    if sub == sz - 1:
                        epool = eA_pool if sz == 3 else eB_pool
                        e = epool.tile([128, sz, 512], BF16, tag="e")
                        nc.scalar.activation(
                            out=e[:, :, 0:qw],
                            in_=s_ps[:, :, 0:qw],
                            func=mybir.ActivationFunctionType.Exp,
                            bias=zero_sb[:, 0:1],
                            scale=0.125,
                        )
                        for s_back in range(slot - sub, slot + 1):
                            e_ref[s_back] = (e, s_back - (slot - sub))
                flush_pend(lag=lag)
                pend.append((period[0], attended(kt)))
                period[0] += 1
            pend.append((period[0] - 1, epilogue))

        # ---- emission schedule ----
        # upfront wave: K chunk0 (4 jobs), Q chunk0 (2), V chunks 0-2 (3),
        # spread across the (still free) scores/attended/rowsum psum banks
        # so evacuations overlap and the PE streams back-to-back.
        up_scA = scA_pool.tile([128, 3, 512], F32, tag="s")
        up_scB = scB_pool.tile([128, 2, 512], F32, tag="s")
        up_att = att_ps_pool.tile([128, 512], F32, tag="att")
        up_rs = rs_ps_pool.tile([128, 512], F32, tag="rs")
        up_slots = [
            ("k", 0, 0, up_scA[:, 0, :]),
            ("k", 0, 1, up_scA[:, 1, :]),
            ("k", 0, 2, up_scA[:, 2, :]),
            ("k", 0, 3, up_scB[:, 0, :]),
            ("q", 0, 0, up_scB[:, 1, :]),
            ("v", 0, 0, up_att[:]),
            ("v", 1, 0, up_rs[:]),
        ]
        for kind, a, b, ps in up_slots:
            emit_pa_job(kind, a, b, ps, pa_count[0] % DC)
            pa_count[0] += 1
        emit_inj_job(("q", 0, 1))
        up_scA2 = scA_pool.tile([128, 3, 512], F32, tag="s")
        emit_pa_job("v", 2, 0, up_scA2[:, 0, :], pa_count[0] % DC)
        pa_count[0] += 1

        # iterations: (hp, qoff, qw); last two (qb,hp) pairs split into
        # 256-query halves so Y jobs pipeline into the exp window
        iters = [
            (0, 0, 512),
            (0, 512, 512),
            (1, 0, 512),
            (1, 512, 512),
            (2, 0, 512),
            (2, 512, 512),
            (3, 0, 256),
            (3, 256, 256),
            (3, 512, 256),
            (3, 768, 256),
        ]
        injections = {i: {} for i in range(len(iters))}
        # V chunks 3..15 inject into iteration 0, one per kt (deadline kt+3)
        for j, st in enumerate(range(3, KT)):
            injections[0][j + 1] = inj_thunk(("v", st, 0))
        # K/Q chunk c+1 injects during the two chunk-c 512-iterations
        for c in (1, 2, 3):
            jobs = [("k", c, sb) for sb in range(S // 512)] + [
                ("q", c, q) for q in range(2)
            ]
            halves = [inj_half_thunks(j) for j in jobs]
            # chunk 1 must not crowd iteration 0 (V jobs live there)
            if c == 1:
                sched = {1: halves[0][0], 3: halves[0][1], 5: halves[1][0],
                         6: halves[1][1], 8: halves[2][0], 9: halves[2][1],
                         10: halves[3][0], 11: halves[3][1], 12: halves[4][0],
                         13: halves[4][1], 14: halves[5][0], 15: halves[5][1]}
                injections[1].update(sched)
            else:
                it_a, it_b = 2 * c - 2, 2 * c - 1
                injections[it_a].update(
                    {2: halves[0][0], 4: halves[0][1], 7: halves[1][0],
                     9: halves[1][1], 12: halves[2][0], 14: halves[2][1]}
                )
                injections[it_b].update(
                    {2: halves[3][0], 4: halves[3][1], 7: halves[4][0],
                     9: halves[4][1], 12: halves[5][0], 14: halves[5][1]}
                )
        injections[2][1] = load_wo_bo
        # Y jobs: rows qt*128; (3, qoff) epilogue completes attended rows
        # qoff..qoff+256 -> Y(qt) two iterations later
        injections[7].update({4: y_thunk(0), 10: y_thunk(1)})
        injections[8].update({4: y_thunk(2), 10: y_thunk(3)})
        injections[9].update({4: y_thunk(4), 10: y_thunk(5)})

        for i, (hp, qoff, qw) in enumerate(iters):
            last = i == len(iters) - 1
            emit_attention(hp, qoff, qw, injections.get(i), lag=1 if last else 3)
        flush_pend()
        emit_y(6)
        emit_y(7)


_BUILT = None


def _build():
    global _BUILT
    if _BUILT is None:
        nc = bacc.Bacc(
            "TRN2", target_bir_lowering=False, debug=False, num_devices=N_CORES
        )
        with tile.TileContext(nc) as tc:
            _kernel_body(tc)
        nc.compile()
        _BUILT = nc
    return _BUILT


def _prepare_in_maps(text_embeds, Wq, bq, Wk, bk, Wv, bv, Wo, bo):
    import ml_dtypes

    bf16 = ml_dtypes.bfloat16
    text_embeds = np.asarray(text_embeds, np.float32)
    Wq = np.ascontiguousarray(np.asarray(Wq, np.float32).astype(bf16))
    Wk = np.ascontiguousarray(np.asarray(Wk, np.float32).astype(bf16))
    Wv = np.ascontiguousarray(np.asarray(Wv, np.float32).astype(bf16))
    Wo32 = np.asarray(Wo, np.float32)
    Wo = np.ascontiguousarray(Wo32.astype(bf16))
    bq = np.ascontiguousarray(np.asarray(bq, np.float32))
    bo2 = (
        np.asarray(bo, np.float64)
        + np.asarray(bv, np.float64) @ Wo32.astype(np.float64)
    ).astype(np.float32)
    in_maps = []
    for core in range(N_CORES):
        b, half = divmod(core, 2)
        xt = text_embeds[b].T  # [D, S]
        if half:
            xt = np.roll(xt, -SQ, axis=1)
        xt = np.ascontiguousarray(xt.astype(bf16))
        in_maps.append(
            {
                "xt": xt,
                "wq": Wq,
                "wk": Wk,
                "wv": Wv,
                "bq": bq,
                "wo": Wo,
                "bo2": bo2,
            }
        )
    return in_maps


def _assemble(results):
    out = np.empty((B, S, VD), np.float32)
    for core in range(N_CORES):
        b, half = divmod(core, 2)
        out[b, half * SQ : (half + 1) * SQ] = results[core]["y"]
    return out


def run(trace=False, **inputs):
    nc = _build()
    in_maps = _prepare_in_maps(**inputs)
    res = bass_utils.run_bass_kernel_spmd(
        nc, in_maps, core_ids=list(range(N_CORES)), trace=trace
    )
    return _assemble(res.results), res


def kernel(**inputs):
    out, _ = run(trace=False, **inputs)
    return out


# revision 19
# speedup vs baseline: 1.0185x; 1.0185x over previous
"""Multi-head self-attention + projector, Trainium2 Bass kernel, 8 NeuronCores.

Reference computation (per batch b):
    Q = X @ Wq + bq; K = X @ Wk + bk; V = X @ Wv + bv      (X: [S, D])
    per head h: P_h = softmax(Q_h K_h^T / sqrt(dh)); A_h = P_h V_h
    Y = concat_h(A_h) @ Wo + bo
Sharding: core i handles batch i//2, query rows (i%2)*1024 .. +1024.
K/V are computed for the full sequence on each core (no collectives).
The host rolls each core's query columns to the front of X^T so a single
SPMD program serves all 8 cores.

Algebraic simplifications (exact w.r.t. softmax):
  - bk dropped (per-query constant in scores, softmax cancels it)
  - bv folded into the output bias on host (softmax rows sum to 1)
  - no max-subtraction in softmax (scores are O(1) for these inputs)

The kernel is exp-throughput bound on the Scalar engine (ACT cost
(N+352)/1.2 ns), so the design maximizes ACT batch size and hides all
other work under the exp stream:
  - scores PSUM alternates a 3-bank and a 2-bank tile; each tile is
    exp'd by ONE ACTIVATE (N=1536/1024) -> 1088 ns/kt vs 1147 at N=1024.
  - attended pair is col-packed: h0 -> psum partitions 0:64 at
    tile_position (0,0), h1 -> 64:128 at (0,64), concurrently, in ONE
    bank.  Only the first matmul into the bank uses start=True (start
    clears has_written for the whole bank; later matmuls overwrite
    where unwritten, accumulate where written).
  - softmax row-sums via M=1 ones-matmuls, 4-way col-packed
    (positions 0/32/64/96) every 2 kt into one more bank.
  - PSUM: 3 (scores A) + 2 (scores B) + 1 (attended) + 1 (rowsum)
    + 1 (injected projection/output jobs) = 8 banks.
  - warmup matmuls on a zeroed tile during the input-DMA window keep
    the PE HAM clock-gate at 8/8 before real work lands.
  - phase A: only K-chunk0 / Q-chunk0 / V0..2 run up front; V3..15
    inject into iteration 1, K/Q chunks 1-3 trickle into later
    iterations (one chunk ahead of first use).
  - last two (qb,hp) iterations run as 256-query halves so the final
    Y = attended @ Wo jobs and their DMAs pipeline into the exp window.
"""

import numpy as np

import concourse.bass as bass
import concourse.mybir as mybir
import concourse.tile as tile
from concourse import bacc, bass_utils

F32 = mybir.dt.float32
BF16 = mybir.dt.bfloat16

B, S, D, HID, HEADS, DH, VD = 4, 2048, 768, 512, 8, 64, 768
N_CORES = 8
SQ = S // 2  # query rows per core
DC = D // 128  # 6 contraction chunks for the projections
HC = HID // 128  # 4 hidden chunks
KT = S // 128  # 16 key chunks
# per-iteration exp batching: 32 (kt, head) slots -> psum tiles of 3,2,3,...,2
BATCH_SIZES = [3, 2] * 6 + [2]
SLOT_MAP = []  # slot -> (batch_idx, sub_idx)
for _b, _sz in enumerate(BATCH_SIZES):
    for _i in range(_sz):
        SLOT_MAP.append((_b, _i))
assert len(SLOT_MAP) == 2 * KT


def _kernel_body(tc):
    nc = tc.nc
    xt_d = nc.dram_tensor("xt", [D, S], BF16, kind="ExternalInput").ap()
    wq_d = nc.dram_tensor("wq", [D, HID], BF16, kind="ExternalInput").ap()
    wk_d = nc.dram_tensor("wk", [D, HID], BF16, kind="ExternalInput").ap()
    wv_d = nc.dram_tensor("wv", [D, HID], BF16, kind="ExternalInput").ap()
    bq_d = nc.dram_tensor("bq", [HID], F32, kind="ExternalInput").ap()
    wo_d = nc.dram_tensor("wo", [HID, VD], BF16, kind="ExternalInput").ap()
    bo_d = nc.dram_tensor("bo2", [VD], F32, kind="ExternalInput").ap()
    y_d = nc.dram_tensor("y", [SQ, VD], F32, kind="ExternalOutput").ap()

    with (
        tc.tile_pool(name="persist", bufs=1) as persist,
        tc.tile_pool(name="scA", bufs=1, space="PSUM") as scA_pool,
        tc.tile_pool(name="scB", bufs=1, space="PSUM") as scB_pool,
        tc.tile_pool(name="att_ps", bufs=1, space="PSUM") as att_ps_pool,
        tc.tile_pool(name="rs_ps", bufs=1, space="PSUM") as rs_ps_pool,
        tc.tile_pool(name="inj_ps", bufs=1, space="PSUM") as inj_ps_pool,
        tc.tile_pool(name="pa_sbuf", bufs=1) as pa_sbuf,
        tc.tile_pool(name="eA_pool", bufs=4) as eA_pool,
        tc.tile_pool(name="eB_pool", bufs=4) as eB_pool,
        tc.tile_pool(name="rb_pool", bufs=2) as rb_pool,
        tc.tile_pool(name="tmp_pool", bufs=2) as tmp_pool,
        tc.tile_pool(name="y_sb", bufs=2) as y_sb_pool,
    ):
        # ---- persistent SBUF tensors ----
        wo_sb = persist.tile([128, HC, VD], BF16)
        bo_sb = persist.tile([128, VD], F32)
        bq_sb = persist.tile([128, HC], F32)
        qt_sb = persist.tile([128, HC, SQ], BF16)
        kt_sb = persist.tile([128, HC, S], BF16)
        v_sb = persist.tile([128, KT, HEADS, DH], BF16)
        att_sb = persist.tile([128, HC, SQ], BF16)
        zero_sb = persist.tile([128, 1], F32)
        ones_sb = persist.tile([128, 1], BF16)
        warm_sb = persist.tile([128, 128], BF16)

        nc.vector.memset(zero_sb[:], 0.0)
        nc.vector.memset(ones_sb[:], 1.0)
        nc.vector.memset(warm_sb[:], 0.0)
        nc.sync.dma_start(out=bq_sb[:], in_=bq_d.rearrange("(c p) -> p c", c=HC))

        # ---- PE warmup: keep the HAM clock-gate busy during input DMA ----
        # ~31 cold matmuls warm the clock (3.4us), the rest hold it at 8/8
        # until the first input chunks land (~12us).
        warm_ps = inj_ps_pool.tile([128, 512], F32, tag="inj")
        for i in range(170):
            nc.tensor.matmul(
                warm_ps[:, 0:128],
                warm_sb[:],
                warm_sb[:],
                start=True,
                stop=True,
            )

        xt_sb = pa_sbuf.tile([128, DC, S], BF16)
        wq_sb = pa_sbuf.tile([128, DC, HID], BF16)
        wk_sb = pa_sbuf.tile([128, DC, HID], BF16)
        wv_sb = pa_sbuf.tile([128, DC, HID], BF16)

        # input DMAs: xt on sync, weights split across gpsimd/vector so the
        # (exp-saturated) scalar queue never issues transfers
        xt_r = xt_d.rearrange("(c p) s -> c p s", c=DC)
        dma_engines = [nc.gpsimd, nc.sync, nc.gpsimd]
        for c in range(DC):
            nc.sync.dma_start(out=xt_sb[:, c, :], in_=xt_r[c])
            for e_i, (w_sb, w_d) in enumerate(
                ((wk_sb, wk_d), (wq_sb, wq_d), (wv_sb, wv_d))
            ):
                w_r = w_d.rearrange("(c p) h -> c p h", c=DC)
                dma_engines[e_i].dma_start(out=w_sb[:, c, :], in_=w_r[c])

        def load_wo_bo():
            for c in range(HC):
                nc.sync.dma_start(
                    out=wo_sb[:, c, :],
                    in_=wo_d.rearrange("(c p) v -> c p v", c=HC)[c],
                )
            bo_row = rb_pool.tile([1, VD], F32, tag="bo_row")
            nc.sync.dma_start(out=bo_row[0:1, :], in_=bo_d[None, :])
            nc.gpsimd.partition_broadcast(bo_sb[:], bo_row[0:1, :])

        # ---- phase A jobs (QKV projections), one [128,512] psum bank each ----
        def emit_pa_job(kind, a, b, ps, off, d_lo=0, d_hi=DC):
            for i in range(d_lo, d_hi):
                d = (off + i) % DC
                if kind == "q":
                    lhsT = wq_sb[:, d, a * 128 : (a + 1) * 128]
                    rhs = xt_sb[:, d, b * 512 : (b + 1) * 512]
                elif kind == "k":
                    lhsT = wk_sb[:, d, a * 128 : (a + 1) * 128]
                    rhs = xt_sb[:, d, b * 512 : (b + 1) * 512]
                else:
                    lhsT = xt_sb[:, d, a * 128 : (a + 1) * 128]
                    rhs = wv_sb[:, d, :]
                nc.tensor.matmul(ps, lhsT, rhs, start=(i == 0), stop=(i == DC - 1))
            if d_hi < DC:
                return
            if kind == "q":
                nc.vector.tensor_scalar_add(
                    out=qt_sb[:, a, b * 512 : (b + 1) * 512],
                    in0=ps,
                    scalar1=bq_sb[:, a : a + 1],
                )
            elif kind == "k":
                nc.vector.tensor_copy(
                    out=kt_sb[:, a, b * 512 : (b + 1) * 512], in_=ps
                )
            else:
                nc.vector.tensor_copy(
                    out=v_sb[:, a, :, :],
                    in_=ps.rearrange("p (h d) -> p h d", h=HEADS),
                )

        pa_count = [0]

        def emit_inj_job(job):
            ps = inj_ps_pool.tile([128, 512], F32, tag="inj")
            emit_pa_job(*job, ps[:], pa_count[0] % DC)
            pa_count[0] += 1

        def inj_thunk(job):
            def thunk():
                with tc.high_priority(offset=-60):
                    emit_inj_job(job)

            return thunk

        def inj_half_thunks(job):
            # one job as two 3-matmul halves sharing a psum tile, so each
            # injection point displaces scores by less than the PE slack
            state = {}

            def first():
                with tc.high_priority(offset=-60):
                    state["ps"] = inj_ps_pool.tile([128, 512], F32, tag="inj", name="inj_ps")
                    state["off"] = pa_count[0] % DC
                    pa_count[0] += 1
                    emit_pa_job(*job, state["ps"][:], state["off"], 0, DC // 2)

            def second():
                with tc.high_priority(offset=-60):
                    emit_pa_job(*job, state["ps"][:], state["off"], DC // 2, DC)

            return first, second

        # ---- phase C job: Y[qt] = attended^T.T @ Wo + bo, via the inject bank
        def emit_y(qt_i):
            y_sb = y_sb_pool.tile([128, VD], F32, tag="ysb")
            y_ps = inj_ps_pool.tile([128, 512], F32, tag="inj")
            for c in range(HC):
                lhsT = att_sb[:, c, qt_i * 128 : (qt_i + 1) * 128]
                nc.tensor.matmul(
                    y_ps[:], lhsT, wo_sb[:, c, 0:512], start=(c == 0), stop=(c == HC - 1)
                )
            nc.vector.tensor_add(y_sb[:, 0:512], y_ps[:], bo_sb[:, 0:512])
            y_ps2 = inj_ps_pool.tile([128, 512], F32, tag="inj")
            for c in range(HC):
                lhsT = att_sb[:, c, qt_i * 128 : (qt_i + 1) * 128]
                nc.tensor.matmul(
                    y_ps2[:, 0 : VD - 512],
                    lhsT,
                    wo_sb[:, c, 512:VD],
                    start=(c == 0),
                    stop=(c == HC - 1),
                )
            nc.vector.tensor_add(
                y_sb[:, 512:VD], y_ps2[:, 0 : VD - 512], bo_sb[:, 512:VD]
            )
            nc.sync.dma_start(
                out=y_d.rearrange("(t p) v -> t p v", p=128)[qt_i], in_=y_sb[:]
            )

        def y_thunk(qt_i):
            def thunk():
                with tc.high_priority(offset=-60):
                    emit_y(qt_i)

            return thunk

        # ---- phase B attention iteration ----
        # attended matmuls / rowsums / epilogue are deferred (pend) so the
        # next scores+exp always lead on the PE/ACT streams.
        pend = []
        period = [0]

        def flush_pend(lag=0):
            while pend and pend[0][0] <= period[0] - lag:
                pend.pop(0)[1]()

        def emit_attention(hp, qoff, qw, inject=None, lag=3):
            h0, h1 = 2 * (hp % HEADS), 2 * (hp % HEADS) + 1
            qs = qt_sb[:, hp, qoff : qoff + qw]
            att_ps = [None]
            rs_ps = [None]
            e_ref = {}  # slot -> (e_tile, sub_idx)
            s_tiles = {}

            def window(w):
                # attended + rowsums for kts (2w, 2w+1), emitted together so
                # the PE sees long same-tiling-mode runs: 4 col64 attended
                # matmuls, then one col32 rowsum quad.  Mode transitions
                # (row<->col) cost ~100-300ns of drain each, so batching them
                # at 2-kt granularity halves the transition tax.
                def thunk():
                    kta, ktb = 2 * w, 2 * w + 1
                    if w == 0:
                        att_ps[0] = att_ps_pool.tile([128, 512], F32, tag="att", name="att_ps")
                        rs_ps[0] = rs_ps_pool.tile([128, 512], F32, tag="rs", name="rs_ps")
                        # zero the banks, then accumulate with start=False
                        # throughout: correct whatever the has_written bits
                        # are (set -> 0+x, clear -> overwrite), and CoreSim's
                        # accumulate-onto-current model agrees.
                        nc.vector.memset(att_ps[0][:], 0.0)
                        nc.vector.memset(rs_ps[0][:], 0.0)
                    att = att_ps[0]
                    rs = rs_ps[0]
                    for kt in (kta, ktb):
                        eh0, i0 = e_ref[2 * kt]
                        eh1, i1 = e_ref[2 * kt + 1]
                        # col-packed pair: h0 -> partitions 0:64 (tile 0,0),
                        # h1 -> 64:128 (tile 0,64), concurrently.
                        nc.tensor.matmul(
                            att[0:DH, 0:qw],
                            v_sb[:, kt, h0, :],
                            eh0[:, i0, 0:qw],
                            start=False,
                            stop=(kt == KT - 1),
                            skip_group_check=True,
                        )
                        nc.tensor.matmul(
                            att[DH:128, 0:qw],
                            v_sb[:, kt, h1, :],
                            eh1[:, i1, 0:qw],
                            start=False,
                            stop=(kt == KT - 1),
                            skip_group_check=True,
                        )
                    quads = [
                        (0, e_ref[4 * w]),       # h0, even kt
                        (32, e_ref[4 * w + 1]),  # h1, even kt
                        (64, e_ref[4 * w + 2]),  # h0, odd kt
                        (96, e_ref[4 * w + 3]),  # h1, odd kt
                    ]
                    for pos, (et, ei) in quads:
                        nc.tensor.matmul(
                            rs[pos : pos + 1, 0:qw],
                            ones_sb[:],
                            et[:, ei, 0:qw],
                            start=False,
                            stop=(w == KT // 2 - 1),
                            skip_group_check=True,
                            tile_position=(0, pos),
                        )

                return thunk

            def epilogue():
                att = att_ps[0]
                rs = rs_ps[0]
                # gather the 4 rowsum partials onto partition 0, stacked on
                # the free axis: g[0] = pos0(h0 even), g[1] = pos32(h1 even),
                # g[2] = pos64(h0 odd), g[3] = pos96(h1 odd) so the strided
                # add pairs (h0e+h0o, h1e+h1o)
                rs_sb = tmp_pool.tile([97, 512], F32, tag="rs_sb")
                nc.vector.tensor_copy(rs_sb[0:97, 0:qw], rs[0:97, 0:qw])
                g = tmp_pool.tile([1, 4, 512], F32, tag="g")
                nc.sync.dma_start(g[0:1, 0, 0:qw], rs_sb[0:1, 0:qw])
                nc.gpsimd.dma_start(g[0:1, 1, 0:qw], rs_sb[32:33, 0:qw])
                nc.sync.dma_start(g[0:1, 2, 0:qw], rs_sb[64:65, 0:qw])
                nc.gpsimd.dma_start(g[0:1, 3, 0:qw], rs_sb[96:97, 0:qw])
                # rec[0,0,:] = 1/rowsum_h0, rec[0,1,:] = 1/rowsum_h1
                rec = rb_pool.tile([1, 2, 512], F32, tag="rec")
                nc.vector.tensor_add(
                    rec[0:1, :, 0:qw], g[0:1, 0:2, 0:qw], g[0:1, 2:4, 0:qw]
                )
                nc.vector.reciprocal_approx_fast(rec[0:1, :, 0:qw], rec[0:1, :, 0:qw])
                rb = rb_pool.tile([128, 512], F32, tag="rb")
                rbB = rb_pool.tile([64, 512], F32, tag="rbB")
                nc.gpsimd.partition_broadcast(rb[0:DH, 0:qw], rec[0:1, 0, 0:qw])
                nc.gpsimd.partition_broadcast(rbB[0:DH, 0:qw], rec[0:1, 1, 0:qw])
                # partition-shift the h1 reciprocals into rb's top half
                nc.sync.dma_start(rb[DH:128, 0:qw], rbB[0:DH, 0:qw])
                nc.vector.tensor_mul(
                    att_sb[:, hp, qoff : qoff + qw], att[:, 0:qw], rb[:, 0:qw]
                )

            for kt in range(KT):
                if inject and kt in inject:
                    inject[kt]()
                ks = kt_sb[:, hp, kt * 128 : (kt + 1) * 128]
                for h in (0, 1):
                    slot = 2 * kt + h
                    b_idx, sub = SLOT_MAP[slot]
                    sz = BATCH_SIZES[b_idx]
                    if sub == 0:
                        pool = scA_pool if sz == 3 else scB_pool
                        s_tiles[b_idx] = pool.tile(
                            [128, sz, 512], F32, tag="s", name="s_ps"
                        )
                    s_ps = s_tiles[b_idx]
                    nc.tensor.matmul(
                        s_ps[:, sub, 0:qw],
                        ks[64 * h : 64 * h + 64, :],
                        qs[64 * h : 64 * h + 64, :],
                        start=True,
                        stop=True,
                    )
                    if sub == sz - 1:
                        epool = eA_pool if sz == 3 else eB_pool
                        e = epool.tile([128, sz, 512], BF16, tag="e")
                        nc.scalar.activation(
                            out=e[:, :, 0:qw],
                            in_=s_ps[:, :, 0:qw],
                            func=mybir.ActivationFunctionType.Exp,
                            bias=zero_sb[:, 0:1],
                            scale=0.125,
                        )
                        for s_back in range(slot - sub, slot + 1):
                            e_ref[s_back] = (e, s_back - (slot - sub))
                if kt % 2 == 1:
                    flush_pend(lag=lag)
                    pend.append((period[0], window(kt // 2)))
                    period[0] += 1
            pend.append((period[0] - 1, epilogue))

        # ---- emission schedule ----
        # upfront wave: K chunk0 (4 jobs), Q chunk0 (2), V chunks 0-2 (3),
        # spread across the (still free) scores/attended/rowsum psum banks
        # so evacuations overlap and the PE streams back-to-back.
        up_scA = scA_pool.tile([128, 3, 512], F32, tag="s")
        up_scB = scB_pool.tile([128, 2, 512], F32, tag="s")
        up_att = att_ps_pool.tile([128, 512], F32, tag="att")
        up_rs = rs_ps_pool.tile([128, 512], F32, tag="rs")
        up_slots = [
            ("k", 0, 0, up_scA[:, 0, :]),
            ("k", 0, 1, up_scA[:, 1, :]),
            ("k", 0, 2, up_scA[:, 2, :]),
            ("k", 0, 3, up_scB[:, 0, :]),
            ("q", 0, 0, up_scB[:, 1, :]),
            ("v", 0, 0, up_att[:]),
            ("v", 1, 0, up_rs[:]),
        ]
        for kind, a, b, ps in up_slots:
            emit_pa_job(kind, a, b, ps, pa_count[0] % DC)
            pa_count[0] += 1
        emit_inj_job(("q", 0, 1))
        up_scA2 = scA_pool.tile([128, 3, 512], F32, tag="s")
        emit_pa_job("v", 2, 0, up_scA2[:, 0, :], pa_count[0] % DC)
        pa_count[0] += 1

        # iterations: (hp, qoff, qw); last two (qb,hp) pairs split into
        # 256-query halves so Y jobs pipeline into the exp window
        iters = [
            (0, 0, 512),
            (0, 512, 512),
            (1, 0, 512),
            (1, 512, 512),
            (2, 0, 512),
            (2, 512, 512),
            (3, 0, 256),
            (3, 256, 256),
            (3, 512, 256),
            (3, 768, 256),
        ]
        injections = {i: {} for i in range(len(iters))}
        # V chunks 3..15 inject into iteration 0, one per kt (deadline kt+3)
        for j, st in enumerate(range(3, KT)):
            injections[0][j + 1] = inj_thunk(("v", st, 0))
        # K/Q chunk c+1 injects during the two chunk-c 512-iterations
        for c in (1, 2, 3):
            jobs = [("k", c, sb) for sb in range(S // 512)] + [
                ("q", c, q) for q in range(2)
            ]
            halves = [inj_half_thunks(j) for j in jobs]
            # chunk 1 must not crowd iteration 0 (V jobs live there)
            if c == 1:
                sched = {1: halves[0][0], 3: halves[0][1], 5: halves[1][0],
                         6: halves[1][1], 8: halves[2][0], 9: halves[2][1],
                         10: halves[3][0], 11: halves[3][1], 12: halves[4][0],
                         13: halves[4][1], 14: halves[5][0], 15: halves[5][1]}
                injections[1].update(sched)
            else:
                it_a, it_b = 2 * c - 2, 2 * c - 1
                injections[it_a].update(
                    {2: halves[0][0], 4: halves[0][1], 7: halves[1][0],
                     9: halves[1][1], 12: halves[2][0], 14: halves[2][1]}
                )
                injections[it_b].update(
                    {2: halves[3][0], 4: halves[3][1], 7: halves[4][0],
                     9: halves[4][1], 12: halves[5][0], 14: halves[5][1]}
                )
        injections[2][1] = load_wo_bo
        # Y jobs: rows qt*128; (3, qoff) epilogue completes attended rows
        # qoff..qoff+256 -> Y(qt) two iterations later
        injections[7].update({4: y_thunk(0), 10: y_thunk(1)})
        injections[8].update({4: y_thunk(2), 10: y_thunk(3)})
        injections[9].update({4: y_thunk(4), 10: y_thunk(5)})

        for i, (hp, qoff, qw) in enumerate(iters):
            last = i == len(iters) - 1
            emit_attention(hp, qoff, qw, injections.get(i), lag=1 if last else 2)
        flush_pend()
        emit_y(6)
        emit_y(7)


_BUILT = None


def _build():
    global _BUILT
    if _BUILT is None:
        nc = bacc.Bacc(
            "TRN2", target_bir_lowering=False, debug=False, num_devices=N_CORES
        )
        with tile.TileContext(nc) as tc:
            _kernel_body(tc)
        nc.compile()
        _BUILT = nc
    return _BUILT


def _prepare_in_maps(text_embeds, Wq, bq, Wk, bk, Wv, bv, Wo, bo):
    import ml_dtypes

    bf16 = ml_dtypes.bfloat16
    text_embeds = np.asarray(text_embeds, np.float32)
    Wq = np.ascontiguousarray(np.asarray(Wq, np.float32).astype(bf16))
    Wk = np.ascontiguousarray(np.asarray(Wk, np.float32).astype(bf16))
    Wv = np.ascontiguousarray(np.asarray(Wv, np.float32).astype(bf16))
    Wo32 = np.asarray(Wo, np.float32)
    Wo = np.ascontiguousarray(Wo32.astype(bf16))
    bq = np.ascontiguousarray(np.asarray(bq, np.float32))
    bo2 = (
        np.asarray(bo, np.float64)
        + np.asarray(bv, np.float64) @ Wo32.astype(np.float64)
    ).astype(np.float32)
    in_maps = []
    for core in range(N_CORES):
        b, half = divmod(core, 2)
        xt = text_embeds[b].T  # [D, S]
        if half:
            xt = np.roll(xt, -SQ, axis=1)
        xt = np.ascontiguousarray(xt.astype(bf16))
        in_maps.append(
            {
                "xt": xt,
                "wq": Wq,
                "wk": Wk,
                "wv": Wv,
                "bq": bq,
                "wo": Wo,
                "bo2": bo2,
            }
        )
    return in_maps


def _assemble(results):
    out = np.empty((B, S, VD), np.float32)
    for core in range(N_CORES):
        b, half = divmod(core, 2)
        out[b, half * SQ : (half + 1) * SQ] = results[core]["y"]
    return out


def run(trace=False, **inputs):
    nc = _build()
    in_maps = _prepare_in_maps(**inputs)
    res = bass_utils.run_bass_kernel_spmd(
        nc, in_maps, core_ids=list(range(N_CORES)), trace=trace
    )
    return _assemble(res.results), res


def kernel(**inputs):
    out, _ = run(trace=False, **inputs)
    return out
